# revision 23
# baseline (speedup 1.0000x reference)
"""Multi-head attention Bass/Tile kernel for Trainium2, 8 cores data-parallel.

Shapes (hardcoded): x [8, 1024, 768], Wqkv [768, 2304], bqkv [2304],
Wproj [768, 768], bproj [768].  B=8 batches -> one batch per NeuronCore.

Per-core dataflow (fp16 matmuls except q/k path in fp8e4 DoubleRow):
  qT8/kT8 [d, n]: fp8 DoubleRow, stationary = Wq8/Wk8 k-tile-pair planes,
                  moving = x8 (fp8, k-tile pairs interleaved plane-major).
                  3 MMs per (head, 512-chunk) at ~2 cols/cycle.
                  q bias added by the DVE PSUM->SBUF copy (per-partition
                  scalar); k bias cancels in softmax; v bias folded on host.
  v     [n, c'] : fp16; stationary = xT-tiles, moving = Wv_aug (ones col)
  S^T   [j, i]  : fp8 DoubleRow with a step-0 plane dim on both operands
                  (each plane reads the same 96 d-rows, so the MM computes
                  2*S at 2 cols/cycle); 2 MMs into one [128,1024] 2-bank psum
  expS^T        : ONE ACT exp per (h,j) reading [128,1024], fused scale
                  E^-0.5/2 (halves the doubled S), psum->sbuf fp16
  o_aug^T [d,i] : fp16; stationary = v head cols (96 + ones), moving = expS^T
                  -> row 96 = softmax denominator (colsum)
  normalize     : DVE recip of colsum row; DMA replicates it across
                  partitions (DRAM bounce, step-0 AP); all-SBUF DVE multiply
  out   [i, e]  : fp16; stationary = o_norm^T head tiles, moving = Wproj rows;
                  fp32 out, DMA to DRAM; proj+v biases added on host.
ACT runs ONLY exp; all PSUM->SBUF copies are on DVE.
"""

import numpy as np
import ml_dtypes

import concourse.bass as bass
import concourse.bacc as bacc
import concourse.mybir as mybir
import concourse.tile as tile

B, N, E, H = 8, 1024, 768, 8
D = E // H          # 96
DP = 128            # padded head dim (weight col tile)
DA = D + 1          # 97: head dim + ones column for colsum
NT = N // 128       # 8 token tiles
ET = E // 128       # 6 embedding k-tiles
EP = ET // 2        # 3 embedding k-tile PAIRS (DoubleRow planes)
SCALE = float(E) ** -0.5

F16 = mybir.dt.float16
F32 = mybir.dt.float32
F8 = mybir.dt.float8e4
NP_F8 = ml_dtypes.float8_e4m3
EXP = mybir.ActivationFunctionType.Exp
DR = mybir.MatmulPerfMode.DoubleRow


def dr0(t, rows, js, w):
    """Step-0-plane DoubleRow AP: [rows, 2, w] where both planes alias the
    same data (the MM then computes 2x the plain product)."""
    sl = t[0:rows, js:js + w]
    return bass.AP(tensor=sl.tensor, offset=sl.offset,
                   ap=[list(sl.ap[0]), [0, 2], [1, w]])


def build_program(repeats=1, loop_n=0):
    """loop_n > 0 wraps the body in a hardware For_i loop (timing use)."""
    import contextlib
    import concourse.hw_specs as hw_specs

    # The Tile scheduler orders each engine's instruction FIFO using the
    # cost model.  Measured HW matmul streams run ~1.44x the modeled rate
    # (the model's LDWEIGHTS TODO), which makes the scheduler think the PE
    # is idle during the S/exp phase and emit head-of-line st-slot waits.
    # Inflate PE cycle time during SCHEDULING only (restored right after)
    # so the static order matches hardware reality.
    import os
    _factor = float(os.environ.get("SCHED_PE_FACTOR", "1.9"))
    _orig_pe_cycle = hw_specs.TRN2Spec.PE_CYCLE
    hw_specs.TRN2Spec.PE_CYCLE = _orig_pe_cycle * _factor
    try:
        nc = _build_program_inner(repeats, loop_n, contextlib)
    finally:
        hw_specs.TRN2Spec.PE_CYCLE = _orig_pe_cycle
    return nc


def _build_program_inner(repeats, loop_n, contextlib):
    nc = bacc.Bacc("TRN2", target_bir_lowering=False)

    xT = nc.dram_tensor("xT", [E, N], F16, kind="ExternalInput")
    x8 = nc.dram_tensor("x8", [EP * 128, 2 * N], F8, kind="ExternalInput")
    wq8 = nc.dram_tensor("wq8", [EP * 128, 2 * H * DP], F8, kind="ExternalInput")
    wk8 = nc.dram_tensor("wk8", [EP * 128, 2 * H * DP], F8, kind="ExternalInput")
    wv = nc.dram_tensor("wv", [E, H * DA], F16, kind="ExternalInput")
    wp = nc.dram_tensor("wp", [E, E], F16, kind="ExternalInput")
    bq = nc.dram_tensor("bq", [DP, H], F32, kind="ExternalInput")
    out = nc.dram_tensor("out", [N, E], F32, kind="ExternalOutput")

    with tile.TileContext(nc) as tc:
        with (
            tc.tile_pool(name="persist", bufs=1) as persist,
            tc.tile_pool(name="exps", bufs=2) as exps,
            tc.tile_pool(name="osb", bufs=2) as osb,
            tc.tile_pool(name="outsb", bufs=2) as outp,
            tc.tile_pool(name="mmps", bufs=2, space="PSUM") as mmps,
            tc.tile_pool(name="stps", bufs=2, space="PSUM") as stps,
            tc.tile_pool(name="dramp", bufs=2, space="DRAM") as dramp,
        ):
            loop_cm = (tc.For_i(0, loop_n, 1,
                                hint_engines=tuple(mybir.ALL_ENGINES))
                       if loop_n > 0 else contextlib.nullcontext())
            with loop_cm:
             for _rep in range(repeats):
                # ---------------- load inputs ----------------
                # DMA order = first-use order: x+wv (v phase) first, x in column
                # chunks so the first v matmuls start as soon as cols land
                x_sb, wv_sb = [], []
                for k in range(ET):
                    xk = persist.tile([128, N], F16, tag=f"x{k}", name=f"x{k}")
                    nc.sync.dma_start(out=xk, in_=xT[k * 128:(k + 1) * 128, :])
                    x_sb.append(xk)
                    vk = persist.tile([128, H * DA], F16, tag=f"wv{k}", name=f"wv{k}")
                    nc.sync.dma_start(out=vk, in_=wv[k * 128:(k + 1) * 128, :])
                    wv_sb.append(vk)
                x8_sb, wq8_sb, wk8_sb = [], [], []
                for t in range(EP):
                    ts = slice(t * 128, (t + 1) * 128)
                    x8t = persist.tile([128, 2 * N], F8, tag=f"x8{t}", name=f"x8{t}")
                    nc.sync.dma_start(out=x8t, in_=x8[ts, :])
                    x8_sb.append(x8t)
                    q8t = persist.tile([128, 2 * H * DP], F8, tag=f"wq8{t}",
                                       name=f"wq8{t}")
                    nc.sync.dma_start(out=q8t, in_=wq8[ts, :])
                    wq8_sb.append(q8t)
                    k8t = persist.tile([128, 2 * H * DP], F8, tag=f"wk8{t}",
                                       name=f"wk8{t}")
                    nc.sync.dma_start(out=k8t, in_=wk8[ts, :])
                    wk8_sb.append(k8t)
                bq_sb = persist.tile([DP, H], F32, tag="bq", name="bq_sb")
                nc.sync.dma_start(out=bq_sb, in_=bq[:, :])
                wp_sb = []
                for h in range(H):
                    ph = persist.tile([D, E], F16, tag=f"wp{h}", name=f"wp{h}")
                    nc.sync.dma_start(out=ph, in_=wp[h * D:(h + 1) * D, :])
                    wp_sb.append(ph)

                # HAM warm-up: keep the PE busy during the initial DMA wait
                # so the first real matmuls run at 2.4 GHz, not the cold
                # 1.2 GHz (the activity window needs ~3.4us of PE work)
                wu = persist.tile([128, 64], F16, tag="wu", name="wu")
                nc.vector.memset(wu, 0.0)
                wu_ps = stps.tile([128, 1024], F32, tag="st", name="wu_ps")
                for _ in range(24):
                    nc.tensor.matmul(wu_ps[0:64, 0:64], wu, wu[:, 0:64],
                                     start=True, stop=True)

                # ---------------- QKV projections ----------------
                qT8 = [persist.tile([D, N], F8, tag=f"qT{c}", name=f"qT{c}")
                       for c in range(H)]
                kT8 = [persist.tile([D, N], F8, tag=f"kT{c}", name=f"kT{c}")
                       for c in range(H)]
                v_sb = [persist.tile([128, H * DA], F16, tag=f"v{n}", name=f"v{n}")
                        for n in range(NT)]

                # v first (needed by every head's AV): stationary = xT n-tile
                for n in range(NT):
                    ns = slice(n * 128, (n + 1) * 128)
                    for off, w in ((0, 512), (512, H * DA - 512)):
                        # own tag: the first v matmul must not inherit a psum-slot
                        # WAR wait on top of its DMA wait (MM allows 1 sync wait)
                        ps = mmps.tile([128, w], F32, tag="mmv", name="ps_v")
                        for k in range(ET):
                            nc.tensor.matmul(
                                ps, x_sb[k][:, ns], wv_sb[k][:, off:off + w],
                                start=(k == 0), stop=(k == ET - 1))
                        nc.vector.tensor_copy(v_sb[n][:, off:off + w], ps)
                    # ones column per head (colsum trick); softmax makes the
                    # k-bias terms cancel and the v-bias is folded on host
                    nc.vector.memset(
                        v_sb[n].rearrange("p (h a) -> p h a", h=H)[:, :, D], 1.0)

                # emit_qk_group(h, idx): one DoubleRow psum accumulation group
                # over the 3 k-tile pairs (idx 0/1 = q chunks, 2/3 = k chunks)
                def emit_qk_group(h, idx):
                    w8, dst = ((wq8_sb, qT8[h]) if idx < 2 else (wk8_sb, kT8[h]))
                    off = (idx % 2) * 512
                    ps = mmps.tile([D, 512], F32, tag="mm", name="ps_qkv")
                    for t in range(EP):
                        w3 = w8[t].rearrange("p (two m) -> p two m", two=2)
                        x3 = x8_sb[t].rearrange("p (two n) -> p two n", two=2)
                        nc.tensor.matmul(
                            ps, w3[:, :, h * DP:h * DP + D],
                            x3[:, :, off:off + 512],
                            start=(t == 0), stop=(t == EP - 1), perf_mode=DR)
                    if idx < 2:
                        nc.vector.tensor_scalar_add(
                            dst[:, off:off + 512], ps, bq_sb[0:D, h:h + 1])
                    else:
                        nc.vector.tensor_copy(dst[:, off:off + 512], ps)

                o_norm = [persist.tile([D, N], F16, tag=f"on{h}", name=f"on{h}")
                          for h in range(H)]

                def emit_av(h, ex):
                    # both AV chunks of head h in borrowed "mmv" psum slots;
                    # off-inner so consecutive MMs share the stationary v tile
                    hs = slice(h * DA, (h + 1) * DA)
                    av0 = mmps.tile([128, 512], F32, tag="mmv", name="av0_ps")
                    av1 = mmps.tile([128, 512], F32, tag="mmv", name="av1_ps")
                    for j in range(NT):
                        nc.tensor.matmul(
                            av0[0:DA, :], v_sb[j][:, hs], ex[j][:, 0:512],
                            start=(j == 0), stop=(j == NT - 1))
                        nc.tensor.matmul(
                            av1[0:DA, :], v_sb[j][:, hs], ex[j][:, 512:1024],
                            start=(j == 0), stop=(j == NT - 1))
                    nc.vector.tensor_copy(o_sb[h][:, 0:512], av0[0:DA, :])
                    nc.vector.tensor_copy(o_sb[h][:, 512:1024], av1[0:DA, :])

                def emit_norm(h, off):
                    if off == 0:
                        rcp[h] = osb.tile([1, N], F16, tag="rcp",
                                          name=f"rcp{h}")
                        with nc.allow_low_precision(reason="denom ~1e3"):
                            nc.vector.reciprocal(rcp[h], o_sb[h][D:DA, :])
                        # replicate the reciprocal row across partitions on
                        # the (idle) DMA engines instead of a PE broadcast
                        # matmul; the mul is then all-SBUF fp16 (DVE 2x mode)
                        rbc[h] = osb.tile([D, N], F16, tag="rbc",
                                          name=f"rbc{h}")
                        # SBUF APs forbid step-0 partitions; bounce the row
                        # through DRAM, whose APs allow broadcast reads
                        dr_t = dramp.tile([1, N], F16, tag="drcp",
                                          name=f"drcp{h}")
                        nc.sync.dma_start(out=dr_t[0:1, :], in_=rcp[h][0:1, :])
                        bcast = bass.AP(
                            tensor=dr_t.tensor, offset=dr_t.offset,
                            ap=[[0, D]] + [list(d) for d in dr_t[0:1, :].ap[1:]])
                        nc.sync.dma_start(out=rbc[h], in_=bcast)
                    nc.vector.tensor_mul(
                        o_norm[h][:, off:off + 512],
                        o_sb[h][0:D, off:off + 512],
                        rbc[h][:, off:off + 512])

                o_sb, rcp, rbc, ex_prev = {}, {}, {}, None
                for idx in range(4):
                    emit_qk_group(0, idx)
                for h in range(H):
                    # S^T+exp for head h; between j-tiles, emit next head's
                    # q/k groups and the PREVIOUS head's AV/norm — the static
                    # PE stream then always has ready matmuls after an
                    # st-slot wait
                    o_sb[h] = osb.tile([DA, N], F16, tag="osb", name=f"osb{h}")
                    ex = []
                    for j in range(NT):
                        exj = exps.tile([128, N], F16, tag=f"ex{j}", name=f"ex{h}_{j}")
                        js = j * 128
                        st = stps.tile([128, 1024], F32, tag="st", name="st_ps")
                        for off in (0, 512):
                            nc.tensor.matmul(
                                st[:, off:off + 512],
                                dr0(kT8[h], D, js, 128),
                                dr0(qT8[h], D, off, 512),
                                start=True, stop=True, perf_mode=DR)
                        # the DR S matmul computes 2*S -> fold the 1/2 into
                        # the exp scale
                        nc.scalar.activation(exj, st, EXP, scale=SCALE / 2)
                        ex.append(exj)
                        if h + 1 < H and j % 2 == 0:
                            emit_qk_group(h + 1, j // 2)
                        if ex_prev is not None:
                            if j == 1:
                                emit_av(h - 1, ex_prev)
                            elif j == 5:
                                emit_norm(h - 1, 0)
                            elif j == 7:
                                emit_norm(h - 1, 512)
                    ex_prev = ex
                # drain the pipeline: last head's AV + norm
                emit_av(H - 1, ex_prev)
                emit_norm(H - 1, 0)
                emit_norm(H - 1, 512)

                # ---------------- output projection ----------------
                for i in range(NT):
                    isl = slice(i * 128, (i + 1) * 128)
                    # off-inner pairs reuse the o_norm stationary; the two
                    # chunks use different tags (mm+mmv) so both psum groups
                    # are live and tag rotation still double-buffers
                    ps0 = mmps.tile([128, 512], F32, tag="mm", name="ps_pr0")
                    ps1 = mmps.tile([128, E - 512], F32, tag="mmv", name="ps_pr1")
                    for h in range(H):
                        nc.tensor.matmul(
                            ps0, o_norm[h][:, isl], wp_sb[h][:, 0:512],
                            start=(h == 0), stop=(h == H - 1))
                        nc.tensor.matmul(
                            ps1, o_norm[h][:, isl], wp_sb[h][:, 512:E],
                            start=(h == 0), stop=(h == H - 1))
                    for ps, off, w in ((ps0, 0, 512), (ps1, 512, E - 512)):
                        osb_t = outp.tile([128, w], F32, tag="out", name="out_sb")
                        nc.vector.tensor_copy(osb_t, ps)
                        nc.sync.dma_start(out=out[isl, off:off + w], in_=osb_t)

    nc.compile()
    return nc


def prep_weights(Wqkv, bqkv, Wproj, bproj):
    Wr = np.asarray(Wqkv, np.float32).reshape(E, H, D, 3)
    br = np.asarray(bqkv, np.float32).reshape(H, D, 3)
    wv_full = np.zeros((E, H * DA), np.float32)
    bq_full = np.zeros((DP, H), np.float32)
    # fp8 q/k weights, k-tile pairs interleaved plane-major:
    # w8[t*128+p, plane*H*DP + h*DP + c] = W[(2t+plane)*128+p, h, c]
    wq8 = np.zeros((EP * 128, 2 * H * DP), np.float32)
    wk8 = np.zeros((EP * 128, 2 * H * DP), np.float32)
    for h in range(H):
        wv_full[:, h * DA:h * DA + D] = Wr[:, h, :, 2]
        bq_full[0:D, h] = br[h, :, 0]
        for t in range(EP):
            for plane in range(2):
                rows = slice((2 * t + plane) * 128, (2 * t + plane + 1) * 128)
                cols = slice(plane * H * DP + h * DP,
                             plane * H * DP + h * DP + D)
                wq8[t * 128:(t + 1) * 128, cols] = Wr[rows, h, :, 0]
                wk8[t * 128:(t + 1) * 128, cols] = Wr[rows, h, :, 1]
    # host-side output bias: attn rows sum to 1, so attn@(v+bv) = attn@v + bv
    # and (o + bv_cat) @ Wproj + bproj = o @ Wproj + bp_eff
    bv_cat = br[:, :, 2].reshape(E)
    bp_eff = bv_cat @ np.asarray(Wproj, np.float64) + np.asarray(bproj, np.float64)
    return {
        "wq8": wq8.astype(NP_F8),
        "wk8": wk8.astype(NP_F8),
        "wv": wv_full.astype(np.float16),
        "wp": np.asarray(Wproj, np.float32).astype(np.float16),
        "bq": bq_full,
    }, bp_eff.astype(np.float32)


def make_in_maps(x, Wqkv, bqkv, Wproj, bproj):
    x = np.asarray(x, np.float32)
    shared, bp_eff = prep_weights(Wqkv, bqkv, Wproj, bproj)
    make_in_maps.bp_eff = bp_eff
    in_maps = []
    for b in range(B):
        xT_b = np.ascontiguousarray(x[b].T)           # [E, N]
        x8_b = np.zeros((EP * 128, 2 * N), np.float32)
        for t in range(EP):
            for plane in range(2):
                rows = slice((2 * t + plane) * 128, (2 * t + plane + 1) * 128)
                x8_b[t * 128:(t + 1) * 128, plane * N:(plane + 1) * N] = xT_b[rows]
        m = {"xT": xT_b.astype(np.float16), "x8": x8_b.astype(NP_F8)}
        m.update(shared)
        in_maps.append(m)
    return in_maps


_prog_cache = []


def kernel(x, Wqkv, bqkv, Wproj, bproj, _run_kwargs=None):
    from concourse.bass_utils import run_bass_kernel_spmd

    in_maps = make_in_maps(x, Wqkv, bqkv, Wproj, bproj)
    if not _prog_cache:
        _prog_cache.append(build_program())
    nc = _prog_cache[0]
    res = run_bass_kernel_spmd(nc, in_maps, core_ids=list(range(B)),
                               **(_run_kwargs or {}))
    out = np.stack([r["out"] for r in res.results], axis=0)
    out = out + make_in_maps.bp_eff
    if _run_kwargs:
        kernel.last_result = res
    return out


# revision 24
# speedup vs baseline: 1.1855x; 1.1855x over previous
"""Multi-head attention Bass/Tile kernel for Trainium2, 8 cores data-parallel.

Shapes (hardcoded): x [8, 1024, 768], Wqkv [768, 2304], bqkv [2304],
Wproj [768, 768], bproj [768].  B=8 batches -> one batch per NeuronCore.

Per-core dataflow (fp16 matmuls except q/k path in fp8e4 DoubleRow):
  qT8/kT8 [d, n]: fp8 DoubleRow, stationary = Wq8/Wk8 k-tile-pair planes,
                  moving = x8 (fp8, k-tile pairs interleaved plane-major).
                  3 MMs per (head, 512-chunk) at ~2 cols/cycle.
                  q bias added by the DVE PSUM->SBUF copy (per-partition
                  scalar); k bias cancels in softmax; v bias folded on host.
  v     [n, c'] : fp16; stationary = xT-tiles, moving = Wv_aug (ones col)
  S^T   [j, i]  : fp8 DoubleRow with a step-0 plane dim on both operands
                  (each plane reads the same 96 d-rows, so the MM computes
                  2*S at 2 cols/cycle); 2 MMs into one [128,1024] 2-bank psum
  expS^T        : ONE ACT exp per (h,j) reading [128,1024], fused scale
                  E^-0.5/2 (halves the doubled S), psum->sbuf fp16
  o_aug^T [d,i] : fp16; stationary = v head cols (96 + ones), moving = expS^T
                  -> row 96 = softmax denominator (colsum)
  normalize     : DVE recip of colsum row; DMA replicates it across
                  partitions (DRAM bounce, step-0 AP); all-SBUF DVE multiply
  out   [i, e]  : fp16; stationary = o_norm^T head tiles, moving = Wproj rows;
                  fp32 out, DMA to DRAM; proj+v biases added on host.
ACT runs ONLY exp; all PSUM->SBUF copies are on DVE.
"""

import numpy as np
import ml_dtypes

import concourse.bass as bass
import concourse.bacc as bacc
import concourse.mybir as mybir
import concourse.tile as tile

B, N, E, H = 8, 1024, 768, 8
D = E // H          # 96
DP = 128            # padded head dim (weight col tile)
DA = D + 1          # 97: head dim + ones column for colsum
NT = N // 128       # 8 token tiles
ET = E // 128       # 6 embedding k-tiles
EP = ET // 2        # 3 embedding k-tile PAIRS (DoubleRow planes)
SCALE = float(E) ** -0.5

F16 = mybir.dt.float16
F32 = mybir.dt.float32
F8 = mybir.dt.float8e4
NP_F8 = ml_dtypes.float8_e4m3
EXP = mybir.ActivationFunctionType.Exp
DR = mybir.MatmulPerfMode.DoubleRow


def dr0(t, rows, js, w):
    """Step-0-plane DoubleRow AP: [rows, 2, w] where both planes alias the
    same data (the MM then computes 2x the plain product)."""
    sl = t[0:rows, js:js + w]
    return bass.AP(tensor=sl.tensor, offset=sl.offset,
                   ap=[list(sl.ap[0]), [0, 2], [1, w]])


def build_program(repeats=1, loop_n=0):
    """loop_n > 0 wraps the body in a hardware For_i loop (timing use)."""
    import contextlib
    import concourse.hw_specs as hw_specs

    # The Tile scheduler orders each engine's instruction FIFO using the
    # cost model.  Measured HW matmul streams run ~1.44x the modeled rate
    # (the model's LDWEIGHTS TODO), which makes the scheduler think the PE
    # is idle during the S/exp phase and emit head-of-line st-slot waits.
    # Inflate PE cycle time during SCHEDULING only (restored right after)
    # so the static order matches hardware reality.
    # Factor sweep measured: 1.0 -> 232us, 1.44 -> 203us, 1.9 -> 244us;
    # 1.44 (fp16-exact) is the sweet spot.
    _orig_pe_cycle = hw_specs.TRN2Spec.PE_CYCLE
    hw_specs.TRN2Spec.PE_CYCLE = _orig_pe_cycle * 1.44
    try:
        nc = _build_program_inner(repeats, loop_n, contextlib)
    finally:
        hw_specs.TRN2Spec.PE_CYCLE = _orig_pe_cycle
    return nc


def _build_program_inner(repeats, loop_n, contextlib):
    nc = bacc.Bacc("TRN2", target_bir_lowering=False)

    xT = nc.dram_tensor("xT", [E, N], F16, kind="ExternalInput")
    x8 = nc.dram_tensor("x8", [EP * 128, 2 * N], F8, kind="ExternalInput")
    wq8 = nc.dram_tensor("wq8", [EP * 128, 2 * H * DP], F8, kind="ExternalInput")
    wk8 = nc.dram_tensor("wk8", [EP * 128, 2 * H * DP], F8, kind="ExternalInput")
    wv = nc.dram_tensor("wv", [E, H * DA], F16, kind="ExternalInput")
    wp = nc.dram_tensor("wp", [E, E], F16, kind="ExternalInput")
    bq = nc.dram_tensor("bq", [DP, H], F32, kind="ExternalInput")
    out = nc.dram_tensor("out", [N, E], F32, kind="ExternalOutput")

    with tile.TileContext(nc) as tc:
        with (
            tc.tile_pool(name="persist", bufs=1) as persist,
            tc.tile_pool(name="exps", bufs=2) as exps,
            tc.tile_pool(name="osb", bufs=2) as osb,
            tc.tile_pool(name="outsb", bufs=2) as outp,
            tc.tile_pool(name="mmps", bufs=2, space="PSUM") as mmps,
            tc.tile_pool(name="stps", bufs=2, space="PSUM") as stps,
            tc.tile_pool(name="dramp", bufs=2, space="DRAM") as dramp,
        ):
            loop_cm = (tc.For_i(0, loop_n, 1,
                                hint_engines=tuple(mybir.ALL_ENGINES))
                       if loop_n > 0 else contextlib.nullcontext())
            with loop_cm:
             for _rep in range(repeats):
                # ---------------- load inputs ----------------
                # DMA order = first-use order: x+wv (v phase) first, x in column
                # chunks so the first v matmuls start as soon as cols land
                x_sb, wv_sb = [], []
                for k in range(ET):
                    xk = persist.tile([128, N], F16, tag=f"x{k}", name=f"x{k}")
                    nc.sync.dma_start(out=xk, in_=xT[k * 128:(k + 1) * 128, :])
                    x_sb.append(xk)
                    vk = persist.tile([128, H * DA], F16, tag=f"wv{k}", name=f"wv{k}")
                    nc.sync.dma_start(out=vk, in_=wv[k * 128:(k + 1) * 128, :])
                    wv_sb.append(vk)
                x8_sb, wq8_sb, wk8_sb = [], [], []
                for t in range(EP):
                    ts = slice(t * 128, (t + 1) * 128)
                    x8t = persist.tile([128, 2 * N], F8, tag=f"x8{t}", name=f"x8{t}")
                    nc.sync.dma_start(out=x8t, in_=x8[ts, :])
                    x8_sb.append(x8t)
                    q8t = persist.tile([128, 2 * H * DP], F8, tag=f"wq8{t}",
                                       name=f"wq8{t}")
                    nc.sync.dma_start(out=q8t, in_=wq8[ts, :])
                    wq8_sb.append(q8t)
                    k8t = persist.tile([128, 2 * H * DP], F8, tag=f"wk8{t}",
                                       name=f"wk8{t}")
                    nc.sync.dma_start(out=k8t, in_=wk8[ts, :])
                    wk8_sb.append(k8t)
                bq_sb = persist.tile([DP, H], F32, tag="bq", name="bq_sb")
                nc.sync.dma_start(out=bq_sb, in_=bq[:, :])
                wp_sb = []
                for h in range(H):
                    ph = persist.tile([D, E], F16, tag=f"wp{h}", name=f"wp{h}")
                    nc.sync.dma_start(out=ph, in_=wp[h * D:(h + 1) * D, :])
                    wp_sb.append(ph)

                # HAM warm-up: keep the PE busy during the initial DMA wait
                # so the first real matmuls run at 2.4 GHz, not the cold
                # 1.2 GHz (the activity window needs ~3.4us of PE work)
                wu = persist.tile([128, 64], F16, tag="wu", name="wu")
                nc.vector.memset(wu, 0.0)
                wu_ps = stps.tile([128, 1024], F32, tag="st", name="wu_ps")
                for _ in range(24):
                    nc.tensor.matmul(wu_ps[0:64, 0:64], wu, wu[:, 0:64],
                                     start=True, stop=True)

                # ---------------- QKV projections ----------------
                qT8 = [persist.tile([D, N], F8, tag=f"qT{c}", name=f"qT{c}")
                       for c in range(H)]
                kT8 = [persist.tile([D, N], F8, tag=f"kT{c}", name=f"kT{c}")
                       for c in range(H)]
                v_sb = [persist.tile([128, H * DA], F16, tag=f"v{n}", name=f"v{n}")
                        for n in range(NT)]

                # v first (needed by every head's AV): stationary = xT n-tile
                for n in range(NT):
                    ns = slice(n * 128, (n + 1) * 128)
                    for off, w in ((0, 512), (512, H * DA - 512)):
                        # own tag: the first v matmul must not inherit a psum-slot
                        # WAR wait on top of its DMA wait (MM allows 1 sync wait)
                        ps = mmps.tile([128, w], F32, tag="mmv", name="ps_v")
                        for k in range(ET):
                            nc.tensor.matmul(
                                ps, x_sb[k][:, ns], wv_sb[k][:, off:off + w],
                                start=(k == 0), stop=(k == ET - 1))
                        nc.vector.tensor_copy(v_sb[n][:, off:off + w], ps)
                    # ones column per head (colsum trick); softmax makes the
                    # k-bias terms cancel and the v-bias is folded on host
                    nc.vector.memset(
                        v_sb[n].rearrange("p (h a) -> p h a", h=H)[:, :, D], 1.0)

                # emit_qk_group(h, idx): one DoubleRow psum accumulation group
                # over the 3 k-tile pairs (idx 0/1 = q chunks, 2/3 = k chunks)
                def emit_qk_group(h, idx):
                    w8, dst = ((wq8_sb, qT8[h]) if idx < 2 else (wk8_sb, kT8[h]))
                    off = (idx % 2) * 512
                    ps = mmps.tile([D, 512], F32, tag="mm", name="ps_qkv")
                    for t in range(EP):
                        w3 = w8[t].rearrange("p (two m) -> p two m", two=2)
                        x3 = x8_sb[t].rearrange("p (two n) -> p two n", two=2)
                        nc.tensor.matmul(
                            ps, w3[:, :, h * DP:h * DP + D],
                            x3[:, :, off:off + 512],
                            start=(t == 0), stop=(t == EP - 1), perf_mode=DR)
                    if idx < 2:
                        nc.vector.tensor_scalar_add(
                            dst[:, off:off + 512], ps, bq_sb[0:D, h:h + 1])
                    else:
                        nc.vector.tensor_copy(dst[:, off:off + 512], ps)

                o_norm = [persist.tile([D, N], F16, tag=f"on{h}", name=f"on{h}")
                          for h in range(H)]

                def emit_av(h, ex):
                    # both AV chunks of head h in borrowed "mmv" psum slots;
                    # off-inner so consecutive MMs share the stationary v tile
                    hs = slice(h * DA, (h + 1) * DA)
                    av0 = mmps.tile([128, 512], F32, tag="mmv", name="av0_ps")
                    av1 = mmps.tile([128, 512], F32, tag="mmv", name="av1_ps")
                    for j in range(NT):
                        nc.tensor.matmul(
                            av0[0:DA, :], v_sb[j][:, hs], ex[j][:, 0:512],
                            start=(j == 0), stop=(j == NT - 1))
                        nc.tensor.matmul(
                            av1[0:DA, :], v_sb[j][:, hs], ex[j][:, 512:1024],
                            start=(j == 0), stop=(j == NT - 1))
                    nc.vector.tensor_copy(o_sb[h][:, 0:512], av0[0:DA, :])
                    nc.vector.tensor_copy(o_sb[h][:, 512:1024], av1[0:DA, :])

                def emit_norm(h, off):
                    if off == 0:
                        rcp[h] = osb.tile([1, N], F16, tag="rcp",
                                          name=f"rcp{h}")
                        with nc.allow_low_precision(reason="denom ~1e3"):
                            nc.vector.reciprocal(rcp[h], o_sb[h][D:DA, :])
                        # replicate the reciprocal row across partitions on
                        # the (idle) DMA engines instead of a PE broadcast
                        # matmul; the mul is then all-SBUF fp16 (DVE 2x mode)
                        rbc[h] = osb.tile([D, N], F16, tag="rbc",
                                          name=f"rbc{h}")
                        # SBUF APs forbid step-0 partitions; bounce the row
                        # through DRAM, whose APs allow broadcast reads
                        dr_t = dramp.tile([1, N], F16, tag="drcp",
                                          name=f"drcp{h}")
                        nc.sync.dma_start(out=dr_t[0:1, :], in_=rcp[h][0:1, :])
                        bcast = bass.AP(
                            tensor=dr_t.tensor, offset=dr_t.offset,
                            ap=[[0, D]] + [list(d) for d in dr_t[0:1, :].ap[1:]])
                        nc.sync.dma_start(out=rbc[h], in_=bcast)
                    nc.vector.tensor_mul(
                        o_norm[h][:, off:off + 512],
                        o_sb[h][0:D, off:off + 512],
                        rbc[h][:, off:off + 512])

                o_sb, rcp, rbc, ex_prev = {}, {}, {}, None
                for idx in range(4):
                    emit_qk_group(0, idx)
                for h in range(H):
                    # S^T+exp for head h; between j-tiles, emit next head's
                    # q/k groups and the PREVIOUS head's AV/norm — the static
                    # PE stream then always has ready matmuls after an
                    # st-slot wait
                    o_sb[h] = osb.tile([DA, N], F16, tag="osb", name=f"osb{h}")
                    ex = []
                    for j in range(NT):
                        exj = exps.tile([128, N], F16, tag=f"ex{j}", name=f"ex{h}_{j}")
                        js = j * 128
                        st = stps.tile([128, 1024], F32, tag="st", name="st_ps")
                        for off in (0, 512):
                            nc.tensor.matmul(
                                st[:, off:off + 512],
                                dr0(kT8[h], D, js, 128),
                                dr0(qT8[h], D, off, 512),
                                start=True, stop=True, perf_mode=DR)
                        # the DR S matmul computes 2*S -> fold the 1/2 into
                        # the exp scale
                        nc.scalar.activation(exj, st, EXP, scale=SCALE / 2)
                        ex.append(exj)
                        if h + 1 < H and j % 2 == 0:
                            emit_qk_group(h + 1, j // 2)
                        if ex_prev is not None:
                            if j == 1:
                                emit_av(h - 1, ex_prev)
                            elif j == 5:
                                emit_norm(h - 1, 0)
                            elif j == 7:
                                emit_norm(h - 1, 512)
                    ex_prev = ex
                # drain the pipeline: last head's AV + norm
                emit_av(H - 1, ex_prev)
                emit_norm(H - 1, 0)
                emit_norm(H - 1, 512)

                # ---------------- output projection ----------------
                for i in range(NT):
                    isl = slice(i * 128, (i + 1) * 128)
                    # off-inner pairs reuse the o_norm stationary; the two
                    # chunks use different tags (mm+mmv) so both psum groups
                    # are live and tag rotation still double-buffers
                    ps0 = mmps.tile([128, 512], F32, tag="mm", name="ps_pr0")
                    ps1 = mmps.tile([128, E - 512], F32, tag="mmv", name="ps_pr1")
                    for h in range(H):
                        nc.tensor.matmul(
                            ps0, o_norm[h][:, isl], wp_sb[h][:, 0:512],
                            start=(h == 0), stop=(h == H - 1))
                        nc.tensor.matmul(
                            ps1, o_norm[h][:, isl], wp_sb[h][:, 512:E],
                            start=(h == 0), stop=(h == H - 1))
                    for ps, off, w in ((ps0, 0, 512), (ps1, 512, E - 512)):
                        osb_t = outp.tile([128, w], F32, tag="out", name="out_sb")
                        nc.vector.tensor_copy(osb_t, ps)
                        nc.sync.dma_start(out=out[isl, off:off + w], in_=osb_t)

    nc.compile()
    return nc


def prep_weights(Wqkv, bqkv, Wproj, bproj):
    Wr = np.asarray(Wqkv, np.float32).reshape(E, H, D, 3)
    br = np.asarray(bqkv, np.float32).reshape(H, D, 3)
    wv_full = np.zeros((E, H * DA), np.float32)
    bq_full = np.zeros((DP, H), np.float32)
    # fp8 q/k weights, k-tile pairs interleaved plane-major:
    # w8[t*128+p, plane*H*DP + h*DP + c] = W[(2t+plane)*128+p, h, c]
    wq8 = np.zeros((EP * 128, 2 * H * DP), np.float32)
    wk8 = np.zeros((EP * 128, 2 * H * DP), np.float32)
    for h in range(H):
        wv_full[:, h * DA:h * DA + D] = Wr[:, h, :, 2]
        bq_full[0:D, h] = br[h, :, 0]
        for t in range(EP):
            for plane in range(2):
                rows = slice((2 * t + plane) * 128, (2 * t + plane + 1) * 128)
                cols = slice(plane * H * DP + h * DP,
                             plane * H * DP + h * DP + D)
                wq8[t * 128:(t + 1) * 128, cols] = Wr[rows, h, :, 0]
                wk8[t * 128:(t + 1) * 128, cols] = Wr[rows, h, :, 1]
    # host-side output bias: attn rows sum to 1, so attn@(v+bv) = attn@v + bv
    # and (o + bv_cat) @ Wproj + bproj = o @ Wproj + bp_eff
    bv_cat = br[:, :, 2].reshape(E)
    bp_eff = bv_cat @ np.asarray(Wproj, np.float64) + np.asarray(bproj, np.float64)
    return {
        "wq8": wq8.astype(NP_F8),
        "wk8": wk8.astype(NP_F8),
        "wv": wv_full.astype(np.float16),
        "wp": np.asarray(Wproj, np.float32).astype(np.float16),
        "bq": bq_full,
    }, bp_eff.astype(np.float32)


def make_in_maps(x, Wqkv, bqkv, Wproj, bproj):
    x = np.asarray(x, np.float32)
    shared, bp_eff = prep_weights(Wqkv, bqkv, Wproj, bproj)
    make_in_maps.bp_eff = bp_eff
    in_maps = []
    for b in range(B):
        xT_b = np.ascontiguousarray(x[b].T)           # [E, N]
        x8_b = np.zeros((EP * 128, 2 * N), np.float32)
        for t in range(EP):
            for plane in range(2):
                rows = slice((2 * t + plane) * 128, (2 * t + plane + 1) * 128)
                x8_b[t * 128:(t + 1) * 128, plane * N:(plane + 1) * N] = xT_b[rows]
        m = {"xT": xT_b.astype(np.float16), "x8": x8_b.astype(NP_F8)}
        m.update(shared)
        in_maps.append(m)
    return in_maps


_prog_cache = []


def kernel(x, Wqkv, bqkv, Wproj, bproj, _run_kwargs=None):
    from concourse.bass_utils import run_bass_kernel_spmd

    in_maps = make_in_maps(x, Wqkv, bqkv, Wproj, bproj)
    if not _prog_cache:
        _prog_cache.append(build_program())
    nc = _prog_cache[0]
    res = run_bass_kernel_spmd(nc, in_maps, core_ids=list(range(B)),
                               **(_run_kwargs or {}))
    out = np.stack([r["out"] for r in res.results], axis=0)
    out = out + make_in_maps.bp_eff
    if _run_kwargs:
        kernel.last_result = res
    return out
